# revision 1
# baseline (speedup 1.0000x reference)
"""Trainium2 Bass kernel for nn_Net_53807350284778 (graph U-Net style
GCN encoder with SAGPool + adjacency augmentation + decoder).

Strategy (8 NeuronCores, SPMD, 4 launches):
  - Node/level row spaces are block-padded: each core owns a fixed-size
    column block of every level's adjacency (stored transposed) and the
    matching row block of features.
  - Host (numpy) does only control-plane work: dense adjacency build,
    top-k selection between launches, index-gathered strip uploads,
    degree/rsqrt vectors. All FLOP-heavy tensor math (augment matmuls
    D@D, GCN aggregations, feature transforms) runs on the cores.
  - Adjacency is bf16 (0/1 exact); features fp32 with float32r matmuls.
  - Cross-core exchange: AllGather collectives (intra-chip, cheap).

Phases:
  ph1: conv1 + score1            -> x1, s1
  ph2: aug1 (P1=(D1@D1>0)) + conv2 + score2 -> T_P1 strips, x2, s2
  ph3: aug2 (P2=(D2@D2>0)) + conv3 + score3 -> T_P2 strips, x3, s3
  ph4: decoder (3 GCNs with unpooling)      -> final [4096, 500] output
"""
import sys

sys.path.insert(0, "/opt/trn_rl_repo")

import numpy as np
import ml_dtypes

import concourse.bass as bass
from concourse import bacc
import concourse.mybir as mybir
import concourse.tile as tile
from concourse.bass_utils import run_bass_kernel_spmd

# ---------------------------------------------------------------- constants
NC = 8
N = 4096
E = 65536
F_IN = 500
H = 64
K1, K2, K3 = 3277, 2622, 2098

B0 = 512            # per-core node block, W0 = 4096
W1, B1 = 3328, 416  # level-1 padded width (26*128), per-core block
W2, B2 = 2688, 336  # level-2 padded width (21*128)

F32 = mybir.dt.float32
F32R = mybir.dt.float32r
BF16 = mybir.dt.bfloat16
BF = ml_dtypes.bfloat16

CORE_IDS = list(range(NC))


def _counts(K, B, nc=NC):
    """Distribute K real entries over nc blocks of capacity B."""
    base = K // nc
    rem = K - base * nc
    cnt = [base + (1 if c < rem else 0) for c in range(nc)]
    assert max(cnt) <= B
    return cnt


CNT1 = _counts(K1, B1)   # [410]*5 + [409]*3
CNT2 = _counts(K2, B2)   # [328]*6 + [327]*2


def _positions(cnt, B):
    """Padded positions (length sum(cnt)) for real entries, rank order."""
    pos = []
    for c, k in enumerate(cnt):
        pos.extend(range(c * B, c * B + k))
    return np.array(pos, dtype=np.int64)


POS1 = _positions(CNT1, B1)  # rank j -> W1 position
POS2 = _positions(CNT2, B2)  # rank t -> W2 position


# ---------------------------------------------------------------- builders
def _load_3d(nc, pool, dram, p, t, f, dt, tag):
    sb = pool.tile([p, t, f], dt, tag=tag)
    nc.sync.dma_start(out=sb[:], in_=dram.ap().rearrange("(t p) f -> p t f", p=p))
    return sb


def _rsqrt_guarded(nc, pool, d_sb, W, tag):
    """dis = where(d>0, 1/sqrt(d), 0) for a [1, W] row in SBUF."""
    m = pool.tile([1, W], F32, tag=tag + "_m")
    nc.vector.tensor_scalar(out=m[:], in0=d_sb[:], scalar1=0.5, scalar2=None,
                            op0=mybir.AluOpType.is_gt)
    dis = pool.tile([1, W], F32, tag=tag + "_dis")
    nc.vector.tensor_scalar_add(dis[:], d_sb[:], 1.0)
    nc.vector.tensor_sub(dis[:], dis[:], m[:])
    nc.vector.reciprocal(dis[:], dis[:])
    nc.scalar.activation(out=dis[:], in_=dis[:],
                         func=mybir.ActivationFunctionType.Sqrt)
    nc.vector.tensor_mul(dis[:], dis[:], m[:])
    return dis


def build_ph1():
    """conv1 (GCN) + score1. Per-core row block R_c = [512c, 512(c+1))."""
    nc = bacc.Bacc("TRN2", target_bir_lowering=False, debug=True)
    KT = N // 128  # 32
    # inputs
    a0t = nc.dram_tensor("a0t", [N, B0], F32R, kind="ExternalInput")
    xts = nc.dram_tensor("xts", [F_IN, B0], F32R, kind="ExternalInput")
    w1 = nc.dram_tensor("w1", [F_IN, H], F32R, kind="ExternalInput")
    b1r = nc.dram_tensor("b1r", [H, 1], F32, kind="ExternalInput")
    b1n = nc.dram_tensor("b1n", [1, H], F32, kind="ExternalInput")
    dis0n = nc.dram_tensor("dis0n", [B0, 1], F32, kind="ExternalInput")
    dis0r = nc.dram_tensor("dis0r", [1, B0], F32, kind="ExternalInput")
    wrel = nc.dram_tensor("wrel", [H, 1], F32R, kind="ExternalInput")
    wroot = nc.dram_tensor("wroot", [H, 1], F32R, kind="ExternalInput")
    brel = nc.dram_tensor("brel", [1, 1], F32, kind="ExternalInput")
    # outputs
    x1n_out = nc.dram_tensor("x1n_out", [B0, H], F32R, kind="ExternalOutput")
    s1_out = nc.dram_tensor("s1_out", [1, B0], F32, kind="ExternalOutput")
    # collective buffers
    cc_xw_in = nc.dram_tensor("cc_xw_in", [B0, H], F32R)
    cc_xw_out = nc.dram_tensor("cc_xw_out", [N, H], F32R, addr_space="Shared")
    cc_x1_in = nc.dram_tensor("cc_x1_in", [B0, H], F32R)
    cc_x1_out = nc.dram_tensor("cc_x1_out", [N, H], F32R, addr_space="Shared")

    with tile.TileContext(nc) as tc:
        with (
            tc.tile_pool(name="cp", bufs=1) as cp,
            tc.tile_pool(name="psh", bufs=2, space="PSUM") as psh,
            tc.tile_pool(name="psc", bufs=1, space="PSUM") as psc,
        ):
            a0t_sb = _load_3d(nc, cp, a0t, 128, KT, B0, F32R, "a0t")
            xts_sb = _load_3d(nc, cp, xts, 125, 4, B0, F32R, "xts")
            w1_sb = _load_3d(nc, cp, w1, 125, 4, H, F32R, "w1")
            b1r_sb = cp.tile([H, 1], F32, tag="b1r")
            nc.sync.dma_start(out=b1r_sb[:], in_=b1r[:])
            b1rep = cp.tile([128, H], F32, tag="b1rep")
            nc.sync.dma_start(out=b1rep[:], in_=b1n.ap().to_broadcast([128, H]))
            dis0n_sb = cp.tile([128, 4, 1], F32, tag="dis0n")
            nc.sync.dma_start(
                out=dis0n_sb[:],
                in_=dis0n.ap().rearrange("(t p) o -> p t o", p=128))
            dis0rep = cp.tile([H, B0], F32, tag="dis0rep")
            nc.sync.dma_start(out=dis0rep[:], in_=dis0r.ap().to_broadcast([H, B0]))
            wrel_sb = cp.tile([H, 1], F32R, tag="wrel")
            nc.sync.dma_start(out=wrel_sb[:], in_=wrel[:])
            wroot_sb = cp.tile([H, 1], F32R, tag="wroot")
            nc.sync.dma_start(out=wroot_sb[:], in_=wroot[:])
            brel_sb = cp.tile([1, 1], F32, tag="brel")
            nc.sync.dma_start(out=brel_sb[:], in_=brel[:])

            # xw1s strip = (dis0 * x)[R_c] @ w1   -> [B0, H]
            xw_sb = cp.tile([128, 4, H], F32R, tag="xw")
            for m in range(4):
                acc = psh.tile([128, H], F32, tag="accN")
                for k in range(4):
                    nc.tensor.matmul(
                        out=acc[:],
                        lhsT=xts_sb[:, k, m * 128:(m + 1) * 128],
                        rhs=w1_sb[:, k, :],
                        start=(k == 0), stop=(k == 3))
                nc.vector.tensor_copy(out=xw_sb[:, m, :], in_=acc[:])
            nc.sync.dma_start(
                out=cc_xw_in.ap().rearrange("(t p) f -> p t f", p=128),
                in_=xw_sb[:])
            nc.gpsimd.collective_compute(
                "AllGather", mybir.AluOpType.bypass,
                replica_groups=[CORE_IDS],
                ins=[cc_xw_in[:]], outs=[cc_xw_out[:]])
            xwf_sb = _load_3d(nc, cp, cc_xw_out, 128, KT, H, F32R, "xwf")

            # conv1 transposed strip: x1Ts = dis0r * (A0 @ xw1s)^T[:, R_c] + b1
            acc_t = psc.tile([H, B0], F32, tag="accT")
            for k in range(KT):
                nc.tensor.matmul(
                    out=acc_t[:], lhsT=xwf_sb[:, k, :],
                    rhs=a0t_sb[:, k, :],
                    start=(k == 0), stop=(k == KT - 1))
            x1ts_sb = cp.tile([H, B0], F32R, tag="x1ts")
            nc.vector.tensor_mul(x1ts_sb[:], acc_t[:], dis0rep[:])
            nc.vector.tensor_tensor(
                out=x1ts_sb[:], in0=x1ts_sb[:],
                in1=b1r_sb[:].to_broadcast([H, B0]), op=mybir.AluOpType.add)

            # conv1 natural strip: x1n = dis0n * (A0 @ xw1s)[R_c] + b1
            x1n_sb = cp.tile([128, 4, H], F32R, tag="x1n")
            for m in range(4):
                acc = psh.tile([128, H], F32, tag="accN")
                for k in range(KT):
                    nc.tensor.matmul(
                        out=acc[:],
                        lhsT=a0t_sb[:, k, m * 128:(m + 1) * 128],
                        rhs=xwf_sb[:, k, :],
                        start=(k == 0), stop=(k == KT - 1))
                nc.vector.tensor_tensor(
                    out=x1n_sb[:, m, :], in0=acc[:],
                    in1=dis0n_sb[:, m, :].to_broadcast([128, H]),
                    op=mybir.AluOpType.mult)
                nc.vector.tensor_add(x1n_sb[:, m, :], x1n_sb[:, m, :], b1rep[:])
            nc.sync.dma_start(
                out=x1n_out.ap().rearrange("(t p) f -> p t f", p=128),
                in_=x1n_sb[:])
            nc.sync.dma_start(
                out=cc_x1_in.ap().rearrange("(t p) f -> p t f", p=128),
                in_=x1n_sb[:])
            nc.gpsimd.collective_compute(
                "AllGather", mybir.AluOpType.bypass,
                replica_groups=[CORE_IDS],
                ins=[cc_x1_in[:]], outs=[cc_x1_out[:]])
            x1f_sb = _load_3d(nc, cp, cc_x1_out, 128, KT, H, F32R, "x1f")

            # score1: y1T = (A0 @ x1)^T[:, R_c]; s1 = wrel^T y1T + wroot^T x1T + brel
            acc_y = psc.tile([H, B0], F32, tag="accT")
            for k in range(KT):
                nc.tensor.matmul(
                    out=acc_y[:], lhsT=x1f_sb[:, k, :],
                    rhs=a0t_sb[:, k, :],
                    start=(k == 0), stop=(k == KT - 1))
            y1t_sb = cp.tile([H, B0], F32R, tag="y1t")
            nc.vector.tensor_copy(out=y1t_sb[:], in_=acc_y[:])
            acc_s = psc.tile([1, B0], F32, tag="accM")
            nc.tensor.matmul(out=acc_s[:], lhsT=wrel_sb[:],
                             rhs=y1t_sb[:], start=True, stop=False)
            nc.tensor.matmul(out=acc_s[:], lhsT=wroot_sb[:],
                             rhs=x1ts_sb[:], start=False, stop=True)
            s1_sb = cp.tile([1, B0], F32, tag="s1")
            nc.vector.tensor_tensor(
                out=s1_sb[:], in0=acc_s[:],
                in1=brel_sb[:].to_broadcast([1, B0]), op=mybir.AluOpType.add)
            nc.sync.dma_start(out=s1_out[:], in_=s1_sb[:])

    nc.compile()
    return nc


def build_aug_phase(W, B, name):
    """aug (P = (D@D > 0)) + conv + score at a pooled level.

    Inputs (per core c):
      u    [B, W]  bf16: D rows of own block (padded rows zero)
      t1   [W, B]  bf16: = u^T (D^T columns of own block)
      xpt  [H, W]  f32 : pooled gated features, transposed, full (replicated)
      w    [H, H], br [H,1], bn [1,H], wrel/wroot [H,1], brel [1,1]
    Outputs:
      tp_out [W, B] bf16: P^T[:, own block]
      xn_out [B, H] f32 : conv output rows (own block, padded-block local)
      s_out  [1, B] f32 : scores row
    """
    nc = bacc.Bacc("TRN2", target_bir_lowering=False, debug=True)
    KT = W // 128
    MT = (B + 127) // 128  # m tiles for the natural strip (last partial)

    u = nc.dram_tensor("u", [B, W], BF16, kind="ExternalInput")
    t1 = nc.dram_tensor("t1", [W, B], BF16, kind="ExternalInput")
    xpt = nc.dram_tensor("xpt", [H, W], F32R, kind="ExternalInput")
    w = nc.dram_tensor("w", [H, H], F32R, kind="ExternalInput")
    br = nc.dram_tensor("br", [H, 1], F32, kind="ExternalInput")
    bn = nc.dram_tensor("bn", [1, H], F32, kind="ExternalInput")
    wrel = nc.dram_tensor("wrel", [H, 1], F32R, kind="ExternalInput")
    wroot = nc.dram_tensor("wroot", [H, 1], F32R, kind="ExternalInput")
    brel = nc.dram_tensor("brel", [1, 1], F32, kind="ExternalInput")

    tp_out = nc.dram_tensor("tp_out", [W, B], F32R, kind="ExternalOutput")
    xn_out = nc.dram_tensor("xn_out", [B, H], F32R, kind="ExternalOutput")
    s_out = nc.dram_tensor("s_out", [1, B], F32, kind="ExternalOutput")

    cc_u_in = nc.dram_tensor("cc_u_in", [B, W], BF16)
    dfull = nc.dram_tensor("dfull", [W, W], BF16, addr_space="Shared")
    cc_d_in = nc.dram_tensor("cc_d_in", [1, B], F32)
    cc_d_out = nc.dram_tensor("cc_d_out", [NC, B], F32, addr_space="Shared")
    cc_x_in = nc.dram_tensor("cc_x_in", [B, H], F32R)
    cc_x_out = nc.dram_tensor("cc_x_out", [W, H], F32R, addr_space="Shared")
    dis_dram = nc.dram_tensor("dis_dram", [1, W], F32)
    diso_dram = nc.dram_tensor("diso_dram", [1, B], F32)

    with tile.TileContext(nc) as tc:
        with (
            tc.tile_pool(name="cp", bufs=1) as cp,
            tc.tile_pool(name="sp", bufs=3) as sp,
            tc.tile_pool(name="psh", bufs=2, space="PSUM") as psh,
            tc.tile_pool(name="psc", bufs=1, space="PSUM") as psc,
        ):
            # ship own D rows, allgather the full D
            nc.sync.dma_start(out=cc_u_in[:], in_=u[:])
            nc.gpsimd.collective_compute(
                "AllGather", mybir.AluOpType.bypass,
                replica_groups=[CORE_IDS],
                ins=[cc_u_in[:]], outs=[dfull[:]])

            t1_sb = _load_3d(nc, cp, t1, 128, KT, B, BF16, "t1")

            # aug: tp[m-tile, :] = (D^T @ D^T)[m rows, own cols] > 0
            # D-full resident in SBUF when it fits (level 2); else stream
            # column panels per m-tile (level 1: 22MB does not fit).
            tp_sb = cp.tile([128, KT, B], F32R, tag="tp")
            # resident D-full needs W*KT*2 bytes/partition; with the f32r
            # tp strip + feature tiles, neither level fits within 208KB -
            # keep the streamed-panel path (threshold left for smaller W).
            resident = (W * KT * 2) <= 64 * 1024
            if resident:
                dful_sb = cp.tile([128, KT, W], BF16, tag="dful")
                nc.sync.dma_start(
                    out=dful_sb[:],
                    in_=dfull.ap().rearrange("(t p) q -> p t q", p=128))
            for m in range(KT):
                if resident:
                    pan = dful_sb[:, :, m * 128:(m + 1) * 128]
                else:
                    pan = sp.tile([128, KT, 128], BF16, tag="pan")
                    nc.sync.dma_start(
                        out=pan[:],
                        in_=dfull.ap()[:, m * 128:(m + 1) * 128]
                        .rearrange("(t p) q -> p t q", p=128))
                acc = psh.tile([128, B], F32, tag="accAug")
                for k in range(KT):
                    nc.tensor.matmul(
                        out=acc[:], lhsT=pan[:, k, :], rhs=t1_sb[:, k, :],
                        start=(k == 0), stop=(k == KT - 1))
                nc.vector.tensor_scalar(
                    out=tp_sb[:, m, :], in0=acc[:], scalar1=0.5, scalar2=None,
                    op0=mybir.AluOpType.is_gt)
            nc.sync.dma_start(
                out=tp_out.ap().rearrange("(t p) b -> p t b", p=128),
                in_=tp_sb[:])

            # degrees of own block: d[b] = colsum_j tp[j, b] = rowsum of P
            ones_f = cp.tile([128, 1], F32, tag="ones_f")
            nc.vector.memset(ones_f[:], 1.0)
            zeros_f = cp.tile([128, H], F32, tag="zeros_f")
            nc.vector.memset(zeros_f[:], 0.0)
            ones_sb = cp.tile([128, 1], F32R, tag="ones")
            nc.vector.tensor_copy(out=ones_sb[:], in_=ones_f[:])
            dacc = psc.tile([1, B], F32, tag="accM")
            for k in range(KT):
                nc.tensor.matmul(out=dacc[:], lhsT=ones_sb[:], rhs=tp_sb[:, k, :],
                                 start=(k == 0), stop=(k == KT - 1))
            drow = cp.tile([1, B], F32, tag="drow")
            nc.vector.tensor_copy(out=drow[:], in_=dacc[:])
            nc.sync.dma_start(out=cc_d_in[:], in_=drow[:])
            nc.gpsimd.collective_compute(
                "AllGather", mybir.AluOpType.bypass,
                replica_groups=[CORE_IDS],
                ins=[cc_d_in[:]], outs=[cc_d_out[:]])
            dfull_sb = cp.tile([1, W], F32, tag="dfull_sb")
            nc.sync.dma_start(out=dfull_sb[:],
                              in_=cc_d_out.ap().rearrange("(o c) b -> o (c b)", o=1))
            dis = _rsqrt_guarded(nc, cp, dfull_sb, W, "g")
            nc.sync.dma_start(out=dis_dram[:], in_=dis[:])
            disrep = cp.tile([H, W], F32, tag="disrep")
            nc.sync.dma_start(out=disrep[:], in_=dis_dram.ap().to_broadcast([H, W]))
            # own-block dis (row + replicated forms)
            diso = _rsqrt_guarded(nc, cp, drow, B, "go")
            nc.sync.dma_start(out=diso_dram[:], in_=diso[:])
            disorep = cp.tile([H, B], F32, tag="disorep")
            nc.sync.dma_start(out=disorep[:],
                              in_=diso_dram.ap().to_broadcast([H, B]))
            ident = cp.tile([1, 1], F32, tag="ident")
            nc.vector.memset(ident[:], 1.0)

            # pooled features -> xw = (dis * xp) @ w  (full, replicated)
            xpt_sb = cp.tile([H, W], F32R, tag="xpt_sb")
            nc.sync.dma_start(out=xpt_sb[:], in_=xpt[:])
            nc.vector.tensor_mul(xpt_sb[:], xpt_sb[:], disrep[:])
            w_sb = cp.tile([H, H], F32R, tag="w_sb")
            nc.sync.dma_start(out=w_sb[:], in_=w[:])
            xw_sb = cp.tile([128, KT, H], F32R, tag="xw_sb")
            for m in range(KT):
                acc = psh.tile([128, H], F32, tag="accN")
                nc.tensor.matmul(
                    out=acc[:], lhsT=xpt_sb[:, m * 128:(m + 1) * 128],
                    rhs=w_sb[:], start=True, stop=True)
                nc.vector.tensor_copy(out=xw_sb[:, m, :], in_=acc[:])

            br_sb = cp.tile([H, 1], F32, tag="br_sb")
            nc.sync.dma_start(out=br_sb[:], in_=br[:])
            bnrep = cp.tile([128, H], F32, tag="bnrep")
            nc.sync.dma_start(out=bnrep[:], in_=bn.ap().to_broadcast([128, H]))

            # conv transposed strip: xTs = disrow_own * (An @ xw)^T[:, own] + b
            acc_t = psc.tile([H, B], F32, tag="accT")
            for k in range(KT):
                nc.tensor.matmul(
                    out=acc_t[:], lhsT=xw_sb[:, k, :],
                    rhs=tp_sb[:, k, :], start=(k == 0), stop=(k == KT - 1))
            xts_sb = cp.tile([H, B], F32R, tag="xts_sb")
            nc.vector.tensor_mul(xts_sb[:], acc_t[:], disorep[:])
            nc.vector.tensor_tensor(
                out=xts_sb[:], in0=xts_sb[:],
                in1=br_sb[:].to_broadcast([H, B]), op=mybir.AluOpType.add)

            # conv natural strip: xn = disn * (An @ xw)[own rows] + b
            xn_sb = cp.tile([128, MT, H], F32R, tag="xn_sb")
            for m in range(MT):
                rows = min(128, B - m * 128)
                acc = psh.tile([128, H], F32, tag="accN")
                for k in range(KT):
                    nc.tensor.matmul(
                        out=acc[:rows, :],
                        lhsT=tp_sb[:, k, m * 128:m * 128 + rows],
                        rhs=xw_sb[:, k, :],
                        start=(k == 0), stop=(k == KT - 1))
                # natural dis for own block from diso (row) via PE transpose
                tp_ps = psc.tile([128, 1], F32, tag="accM")
                nc.tensor.transpose(
                    out=tp_ps[:rows, :], in_=diso[:, m * 128:m * 128 + rows],
                    identity=ident[:])
                dison = sp.tile([128, 1], F32, tag="dison")
                nc.vector.tensor_copy(out=dison[:rows, :], in_=tp_ps[:rows, :])
                if rows < 128:
                    nc.vector.tensor_copy(out=xn_sb[:, m, :], in_=zeros_f[:])
                nc.vector.tensor_tensor(
                    out=xn_sb[:rows, m, :], in0=acc[:rows, :],
                    in1=dison[:rows, :].to_broadcast([rows, H]),
                    op=mybir.AluOpType.mult)
                nc.vector.tensor_add(
                    xn_sb[:rows, m, :], xn_sb[:rows, m, :], bnrep[:rows, :])
            for m in range(MT):
                rows = min(128, B - m * 128)
                nc.sync.dma_start(out=xn_out[m * 128:m * 128 + rows, :],
                                  in_=xn_sb[:rows, m, :])
                nc.sync.dma_start(out=cc_x_in[m * 128:m * 128 + rows, :],
                                  in_=xn_sb[:rows, m, :])
            nc.gpsimd.collective_compute(
                "AllGather", mybir.AluOpType.bypass,
                replica_groups=[CORE_IDS],
                ins=[cc_x_in[:]], outs=[cc_x_out[:]])
            xf_sb = _load_3d(nc, cp, cc_x_out, 128, KT, H, F32R, "xf")

            # score: yT = (P @ x)^T[:, own]; s = wrel^T yT + wroot^T xT + brel
            acc_y = psc.tile([H, B], F32, tag="accT")
            for k in range(KT):
                nc.tensor.matmul(
                    out=acc_y[:], lhsT=xf_sb[:, k, :],
                    rhs=tp_sb[:, k, :], start=(k == 0), stop=(k == KT - 1))
            yt_sb = cp.tile([H, B], F32R, tag="yt_sb")
            nc.vector.tensor_copy(out=yt_sb[:], in_=acc_y[:])
            wrel_sb = cp.tile([H, 1], F32R, tag="wrel_sb")
            nc.sync.dma_start(out=wrel_sb[:], in_=wrel[:])
            wroot_sb = cp.tile([H, 1], F32R, tag="wroot_sb")
            nc.sync.dma_start(out=wroot_sb[:], in_=wroot[:])
            brel_sb = cp.tile([1, 1], F32, tag="brel_sb")
            nc.sync.dma_start(out=brel_sb[:], in_=brel[:])
            acc_s = psc.tile([1, B], F32, tag="accM")
            nc.tensor.matmul(out=acc_s[:], lhsT=wrel_sb[:],
                             rhs=yt_sb[:], start=True, stop=False)
            nc.tensor.matmul(out=acc_s[:], lhsT=wroot_sb[:],
                             rhs=xts_sb[:], start=False, stop=True)
            s_sb = cp.tile([1, B], F32, tag="s_sb")
            nc.vector.tensor_tensor(
                out=s_sb[:], in0=acc_s[:],
                in1=brel_sb[:].to_broadcast([1, B]), op=mybir.AluOpType.add)
            nc.sync.dma_start(out=s_out[:], in_=s_sb[:])

    nc.compile()
    return nc


def build_ph4():
    """Decoder: 3 unpool+GCN steps down to the [4096, 500] output.

    Per-core inputs:
      tp2    [W2, B2] bf16: P2^T[:, own2]
      tp1s2  [W2, B1] bf16: P1[own1 rows, S2-embedded cols]^T (pad rows zero)
      a0s1t  [W1, B0] bf16: A0[R_c rows, S1 cols]^T (pad rows zero)
      up3ts  [H, W2]  f32 : dis2 * gated masked x3^T (full, replicated)
      dis2r  [1, B2], dis1r [1, B1], dis0n [B0, 1]
      dis1s2 [1, W2]  f32 : dis1[r2[t]] row
      dis0s1 [1, W1]  f32 : dis0[r1[j]] row
      u0w/u1w [H, H], u0br/u1br [H, 1], u2w [H, F_IN], u2bn [1, F_IN]
    Output: z_out [B0, F_IN] f32 (own node rows of the final result)
    """
    nc = bacc.Bacc("TRN2", target_bir_lowering=False, debug=True)
    KT2 = W2 // 128  # 21
    KT1 = W1 // 128  # 26
    MT2 = (B2 + 127) // 128
    MT1 = (B1 + 127) // 128

    tp2 = nc.dram_tensor("tp2", [W2, B2], F32R, kind="ExternalInput")
    tp1s2 = nc.dram_tensor("tp1s2", [W2, B1], F32R, kind="ExternalInput")
    a0s1t = nc.dram_tensor("a0s1t", [W1, B0], F32R, kind="ExternalInput")
    up3ts = nc.dram_tensor("up3ts", [H, W2], F32R, kind="ExternalInput")
    dis2r = nc.dram_tensor("dis2r", [1, B2], F32, kind="ExternalInput")
    dis1r = nc.dram_tensor("dis1r", [1, B1], F32, kind="ExternalInput")
    dis0n = nc.dram_tensor("dis0n", [B0, 1], F32, kind="ExternalInput")
    dis1s2 = nc.dram_tensor("dis1s2", [1, W2], F32, kind="ExternalInput")
    dis0s1 = nc.dram_tensor("dis0s1", [1, W1], F32, kind="ExternalInput")
    u0w = nc.dram_tensor("u0w", [H, H], F32R, kind="ExternalInput")
    u0br = nc.dram_tensor("u0br", [H, 1], F32, kind="ExternalInput")
    u1w = nc.dram_tensor("u1w", [H, H], F32R, kind="ExternalInput")
    u1br = nc.dram_tensor("u1br", [H, 1], F32, kind="ExternalInput")
    u2w = nc.dram_tensor("u2w", [H, F_IN], F32R, kind="ExternalInput")
    u2bn = nc.dram_tensor("u2bn", [1, F_IN], F32, kind="ExternalInput")

    z_out = nc.dram_tensor("z_out", [B0, F_IN], F32, kind="ExternalOutput")

    cc_z0_in = nc.dram_tensor("cc_z0_in", [H, B2], F32R)
    cc_z0_out = nc.dram_tensor("cc_z0_out", [NC * H, B2], F32R, addr_space="Shared")
    cc_z1_in = nc.dram_tensor("cc_z1_in", [H, B1], F32R)
    cc_z1_out = nc.dram_tensor("cc_z1_out", [NC * H, B1], F32R, addr_space="Shared")

    with tile.TileContext(nc) as tc:
        with (
            tc.tile_pool(name="cp", bufs=1) as cp,
            tc.tile_pool(name="sp", bufs=3) as sp,
            tc.tile_pool(name="psh", bufs=2, space="PSUM") as psh,
            tc.tile_pool(name="psc", bufs=1, space="PSUM") as psc,
            tc.tile_pool(name="psz", bufs=1, space="PSUM") as psz,
        ):
            tp2_sb = _load_3d(nc, cp, tp2, 128, KT2, B2, F32R, "tp2")
            up3_sb = cp.tile([H, W2], F32R, tag="up3")
            nc.sync.dma_start(out=up3_sb[:], in_=up3ts[:])
            u0w_sb = cp.tile([H, H], F32R, tag="u0w")
            nc.sync.dma_start(out=u0w_sb[:], in_=u0w[:])
            u0br_sb = cp.tile([H, 1], F32, tag="u0br")
            nc.sync.dma_start(out=u0br_sb[:], in_=u0br[:])
            dis2rep = cp.tile([H, B2], F32, tag="dis2rep")
            nc.sync.dma_start(out=dis2rep[:], in_=dis2r.ap().to_broadcast([H, B2]))

            # xwu0 = up3s @ u0w (full, replicated): [W2, H]
            xwu0_sb = cp.tile([128, KT2, H], F32R, tag="xwu0")
            for m in range(KT2):
                acc = psh.tile([128, H], F32, tag="accW")
                nc.tensor.matmul(
                    out=acc[:], lhsT=up3_sb[:, m * 128:(m + 1) * 128],
                    rhs=u0w_sb[:], start=True, stop=True)
                nc.vector.tensor_copy(out=xwu0_sb[:, m, :], in_=acc[:])

            # z0T strip = relu(dis2r * (P2 @ xwu0)^T[:, own2] + u0b)
            acc0 = psc.tile([H, B2], F32, tag="accS")
            for k in range(KT2):
                nc.tensor.matmul(
                    out=acc0[:], lhsT=xwu0_sb[:, k, :],
                    rhs=tp2_sb[:, k, :], start=(k == 0), stop=(k == KT2 - 1))
            z0t_sb = cp.tile([H, B2], F32R, tag="z0t")
            nc.vector.tensor_mul(z0t_sb[:], acc0[:], dis2rep[:])
            nc.vector.tensor_tensor(
                out=z0t_sb[:], in0=z0t_sb[:],
                in1=u0br_sb[:].to_broadcast([H, B2]), op=mybir.AluOpType.add)
            nc.vector.tensor_scalar_max(z0t_sb[:], z0t_sb[:], 0.0)
            nc.sync.dma_start(out=cc_z0_in[:], in_=z0t_sb[:])
            nc.gpsimd.collective_compute(
                "AllGather", mybir.AluOpType.bypass,
                replica_groups=[CORE_IDS],
                ins=[cc_z0_in[:]], outs=[cc_z0_out[:]])
            # z0T full: [H, W2] via [H, c, B2] view
            z0f_sb = cp.tile([H, NC, B2], F32R, tag="z0f")
            nc.sync.dma_start(
                out=z0f_sb[:],
                in_=cc_z0_out.ap().rearrange("(c h) b -> h c b", h=H))
            # scale cols by dis1[S2] row
            d1s2rep = cp.tile([H, W2], F32, tag="d1s2rep")
            nc.sync.dma_start(out=d1s2rep[:], in_=dis1s2.ap().to_broadcast([H, W2]))
            z0fs_sb = z0f_sb[:].rearrange("h c b -> h (c b)")
            nc.vector.tensor_mul(z0fs_sb, z0fs_sb, d1s2rep[:])

            # xwu1 = z0fs^T @ u1w: [W2, H]
            u1w_sb = cp.tile([H, H], F32R, tag="u1w")
            nc.sync.dma_start(out=u1w_sb[:], in_=u1w[:])
            xwu1_sb = cp.tile([128, KT2, H], F32R, tag="xwu1")
            for m in range(KT2):
                acc = psh.tile([128, H], F32, tag="accW")
                nc.tensor.matmul(
                    out=acc[:], lhsT=z0fs_sb[:, m * 128:(m + 1) * 128],
                    rhs=u1w_sb[:], start=True, stop=True)
                nc.vector.tensor_copy(out=xwu1_sb[:, m, :], in_=acc[:])

            # z1T strip = relu(dis1r * (P1 @ up2)^T[:, own1] + u1b)
            # contraction in W2 space against tp1s2 (P1 rows gathered at S2)
            tp1s2_sb = _load_3d(nc, cp, tp1s2, 128, KT2, B1, F32R, "tp1s2")
            u1br_sb = cp.tile([H, 1], F32, tag="u1br")
            nc.sync.dma_start(out=u1br_sb[:], in_=u1br[:])
            dis1rep = cp.tile([H, B1], F32, tag="dis1rep")
            nc.sync.dma_start(out=dis1rep[:], in_=dis1r.ap().to_broadcast([H, B1]))
            acc1 = psc.tile([H, B1], F32, tag="accS")
            for k in range(KT2):
                nc.tensor.matmul(
                    out=acc1[:], lhsT=xwu1_sb[:, k, :],
                    rhs=tp1s2_sb[:, k, :], start=(k == 0), stop=(k == KT2 - 1))
            z1t_sb = cp.tile([H, B1], F32R, tag="z1t")
            nc.vector.tensor_mul(z1t_sb[:], acc1[:], dis1rep[:])
            nc.vector.tensor_tensor(
                out=z1t_sb[:], in0=z1t_sb[:],
                in1=u1br_sb[:].to_broadcast([H, B1]), op=mybir.AluOpType.add)
            nc.vector.tensor_scalar_max(z1t_sb[:], z1t_sb[:], 0.0)
            nc.sync.dma_start(out=cc_z1_in[:], in_=z1t_sb[:])
            nc.gpsimd.collective_compute(
                "AllGather", mybir.AluOpType.bypass,
                replica_groups=[CORE_IDS],
                ins=[cc_z1_in[:]], outs=[cc_z1_out[:]])
            z1f_sb = cp.tile([H, NC, B1], F32R, tag="z1f")
            nc.sync.dma_start(
                out=z1f_sb[:],
                in_=cc_z1_out.ap().rearrange("(c h) b -> h c b", h=H))
            d0s1rep = cp.tile([H, W1], F32, tag="d0s1rep")
            nc.sync.dma_start(out=d0s1rep[:], in_=dis0s1.ap().to_broadcast([H, W1]))
            z1fs_sb = z1f_sb[:].rearrange("h c b -> h (c b)")
            nc.vector.tensor_mul(z1fs_sb, z1fs_sb, d0s1rep[:])

            # xwu2 = z1fs^T @ u2w: [W1, F_IN]
            u2w_sb = cp.tile([H, F_IN], F32R, tag="u2w")
            nc.sync.dma_start(out=u2w_sb[:], in_=u2w[:])
            # final: z rows = dis0n * (A0 @ up0)[R_c] + u2b
            # k-outer with 4 live PSUM banks: produce each xwu2 k-tile on the
            # fly (no DRAM roundtrip) and accumulate into all 4 m-tiles.
            dis0n_sb = cp.tile([128, 4, 1], F32, tag="dis0n_sb")
            nc.sync.dma_start(
                out=dis0n_sb[:],
                in_=dis0n.ap().rearrange("(t p) o -> p t o", p=128))
            u2brep = cp.tile([128, F_IN], F32, tag="u2brep")
            nc.sync.dma_start(out=u2brep[:], in_=u2bn.ap().to_broadcast([128, F_IN]))
            z_sb = cp.tile([128, 4, F_IN], F32, tag="z_sb")
            accz = []
            for m in range(4):
                accz_m = psz.tile([128, F_IN], F32, tag=f"accZ{m}")
                accz.append(accz_m)
            for k in range(KT1):
                accw = psh.tile([128, F_IN], F32, tag="accW")
                nc.tensor.matmul(
                    out=accw[:], lhsT=z1fs_sb[:, k * 128:(k + 1) * 128],
                    rhs=u2w_sb[:], start=True, stop=True)
                xwu2_t = sp.tile([128, F_IN], F32R, tag="xwu2_t")
                nc.vector.tensor_copy(out=xwu2_t[:], in_=accw[:])
                a_t = sp.tile([128, B0], F32R, tag="a_t")
                nc.sync.dma_start(out=a_t[:],
                                  in_=a0s1t[k * 128:(k + 1) * 128, :])
                for m in range(4):
                    nc.tensor.matmul(
                        out=accz[m][:],
                        lhsT=a_t[:, m * 128:(m + 1) * 128],
                        rhs=xwu2_t[:],
                        start=(k == 0), stop=(k == KT1 - 1))
            for m in range(4):
                nc.vector.tensor_tensor(
                    out=z_sb[:, m, :], in0=accz[m][:],
                    in1=dis0n_sb[:, m, :].to_broadcast([128, F_IN]),
                    op=mybir.AluOpType.mult)
                nc.vector.tensor_add(z_sb[:, m, :], z_sb[:, m, :], u2brep[:])
            nc.sync.dma_start(
                out=z_out.ap().rearrange("(t p) f -> p t f", p=128),
                in_=z_sb[:])

    nc.compile()
    return nc


# ---------------------------------------------------------------- host side
_PROGS = {}


def _prog(name):
    if name not in _PROGS:
        if name == "ph1":
            _PROGS[name] = build_ph1()
        elif name == "ph2":
            _PROGS[name] = build_aug_phase(W1, B1, "ph2")
        elif name == "ph3":
            _PROGS[name] = build_aug_phase(W2, B2, "ph3")
        elif name == "ph4":
            _PROGS[name] = build_ph4()
    return _PROGS[name]


def _run(name, in_maps):
    import os
    prog = _prog(name)
    if os.environ.get("KERNEL_SIM"):
        from concourse.bass_interp import MultiCoreSim
        sim = MultiCoreSim(prog, NC)
        for c in range(NC):
            for k, v in in_maps[c].items():
                sim.cores[c].tensor(k)[:] = v
        sim.simulate(check_with_hw=False)
        out_names = []
        for alloc in prog.m.functions[0].allocations:
            if isinstance(alloc, mybir.MemoryLocationSet) and \
                    alloc.kind == "ExternalOutput":
                out_names.append(alloc.memorylocations[0].name)
        return [{k: np.array(sim.cores[c].mem_tensor(k)) for k in out_names}
                for c in range(NC)]
    return run_bass_kernel_spmd(prog, in_maps, CORE_IDS).results


def _f32(a):
    return np.ascontiguousarray(np.asarray(a), dtype=np.float32)


def _bf(a):
    return np.ascontiguousarray(np.asarray(a, dtype=np.float32).astype(BF))


def _guard_rsqrt(d):
    return np.where(d > 0, 1.0 / np.sqrt(np.maximum(d, 1e-30)), 0.0).astype(
        np.float32)


def _topk_sorted(s, k):
    idx = np.argpartition(-s, k - 1)[:k]
    return np.sort(idx)


def kernel(x, w1, b1, w2, b2, w3, b3,
           p1_wrel, p1_brel, p1_wroot,
           p2_wrel, p2_brel, p2_wroot,
           p3_wrel, p3_brel, p3_wroot,
           u0_w, u0_b, u1_w, u1_b, u2_w, u2_b,
           edge_index):
    x = _f32(x)
    ei = np.asarray(edge_index)
    ei = ei.astype(np.int64)

    # dense adjacency with self loops (host, control-plane)
    A0 = np.zeros((N, N), np.float32)
    A0[ei[1], ei[0]] = 1.0
    np.fill_diagonal(A0, 1.0)
    d0 = A0.sum(axis=1)
    dis0 = _guard_rsqrt(d0)

    blocks0 = [slice(c * B0, (c + 1) * B0) for c in range(NC)]

    # ---------------- phase 1: conv1 + score1
    in1 = []
    for c in range(NC):
        rc = blocks0[c]
        in1.append({
            "a0t": _f32(A0[rc, :].T),
            "xts": _f32((x[rc, :] * dis0[rc, None]).T),
            "w1": _f32(w1),
            "b1r": _f32(b1).reshape(H, 1),
            "b1n": _f32(b1).reshape(1, H),
            "dis0n": _f32(dis0[rc]).reshape(B0, 1),
            "dis0r": _f32(dis0[rc]).reshape(1, B0),
            "wrel": _f32(p1_wrel).reshape(H, 1),
            "wroot": _f32(p1_wroot).reshape(H, 1),
            "brel": _f32(p1_brel).reshape(1, 1),
        })
    r1 = _run("ph1", in1)
    x1 = np.concatenate([r1[c]["x1n_out"] for c in range(NC)], axis=0)
    s1 = np.concatenate([r1[c]["s1_out"][0] for c in range(NC)])

    S1 = _topk_sorted(s1, K1)
    gate1 = np.tanh(s1[S1]).astype(np.float32)

    # ---------------- phase 2: aug1 + conv2 + score2
    D1w = np.zeros((W1, W1), np.float32)
    D1w[np.ix_(POS1, POS1)] = A0[np.ix_(S1, S1)]
    x1pT = np.zeros((H, W1), np.float32)
    x1pT[:, POS1] = (x1[S1] * gate1[:, None]).T
    in2 = []
    for c in range(NC):
        blk = slice(c * B1, (c + 1) * B1)
        u_c = D1w[blk, :]
        in2.append({
            "u": _bf(u_c),
            "t1": _bf(u_c.T),
            "xpt": _f32(x1pT),
            "w": _f32(w2),
            "br": _f32(b2).reshape(H, 1),
            "bn": _f32(b2).reshape(1, H),
            "wrel": _f32(p2_wrel).reshape(H, 1),
            "wroot": _f32(p2_wroot).reshape(H, 1),
            "brel": _f32(p2_brel).reshape(1, 1),
        })
    r2 = _run("ph2", in2)
    P1T = np.concatenate(  # P1^T in W1 space
        [np.asarray(r2[c]["tp_out"], dtype=np.float32) for c in range(NC)], axis=1)
    x2w = np.concatenate([r2[c]["xn_out"] for c in range(NC)], axis=0)
    s2w = np.concatenate([r2[c]["s_out"][0] for c in range(NC)])
    s2r = s2w[POS1]
    x2r = x2w[POS1]

    S2 = _topk_sorted(s2r, K2)          # level-1 ranks
    gate2 = np.tanh(s2r[S2]).astype(np.float32)
    P1 = P1T.T
    d1w = P1.sum(axis=1)
    dis1w = _guard_rsqrt(d1w)

    # ---------------- phase 3: aug2 + conv3 + score3
    pos1_s2 = POS1[S2]                   # W1 positions of level-2 entries
    D2w = np.zeros((W2, W2), np.float32)
    D2w[np.ix_(POS2, POS2)] = P1[np.ix_(pos1_s2, pos1_s2)]
    x2pT = np.zeros((H, W2), np.float32)
    x2pT[:, POS2] = (x2r[S2] * gate2[:, None]).T
    in3 = []
    for c in range(NC):
        blk = slice(c * B2, (c + 1) * B2)
        u_c = D2w[blk, :]
        in3.append({
            "u": _bf(u_c),
            "t1": _bf(u_c.T),
            "xpt": _f32(x2pT),
            "w": _f32(w3),
            "br": _f32(b3).reshape(H, 1),
            "bn": _f32(b3).reshape(1, H),
            "wrel": _f32(p3_wrel).reshape(H, 1),
            "wroot": _f32(p3_wroot).reshape(H, 1),
            "brel": _f32(p3_brel).reshape(1, 1),
        })
    r3 = _run("ph3", in3)
    P2T = np.concatenate(
        [np.asarray(r3[c]["tp_out"], dtype=np.float32) for c in range(NC)], axis=1)
    x3w = np.concatenate([r3[c]["xn_out"] for c in range(NC)], axis=0)
    s3w = np.concatenate([r3[c]["s_out"][0] for c in range(NC)])
    s3r = s3w[POS2]
    x3r = x3w[POS2]

    S3 = _topk_sorted(s3r, K3)          # level-2 ranks
    gate3 = np.tanh(s3r[S3]).astype(np.float32)
    P2 = P2T.T
    d2w = P2.sum(axis=1)
    dis2w = _guard_rsqrt(d2w)

    # ---------------- phase 4: decoder
    up3 = np.zeros((W2, H), np.float32)
    up3[POS2[S3]] = x3r[S3] * gate3[:, None]
    up3s_T = (up3 * dis2w[:, None]).T            # [H, W2]

    dis1s2 = np.zeros(W2, np.float32)
    dis1s2[POS2] = dis1w[pos1_s2]
    dis0s1 = np.zeros(W1, np.float32)
    dis0s1[POS1] = dis0[S1]

    Q = np.zeros((W1, W2), np.float32)           # P1[:, S2 embedded]
    Q[:, POS2] = P1[:, pos1_s2]

    in4 = []
    for c in range(NC):
        rc = blocks0[c]
        blk1 = slice(c * B1, (c + 1) * B1)
        blk2 = slice(c * B2, (c + 1) * B2)
        G = np.zeros((W1, B0), np.float32)       # A0[R_c, S1]^T embedded
        G[POS1, :] = A0[np.ix_(range(c * B0, (c + 1) * B0), S1)].T
        in4.append({
            "tp2": _f32(np.asarray(r3[c]["tp_out"], dtype=np.float32)),
            "tp1s2": _f32(Q[blk1, :].T),
            "a0s1t": _f32(G),
            "up3ts": _f32(up3s_T),
            "dis2r": _f32(dis2w[blk2]).reshape(1, B2),
            "dis1r": _f32(dis1w[blk1]).reshape(1, B1),
            "dis0n": _f32(dis0[rc]).reshape(B0, 1),
            "dis1s2": _f32(dis1s2).reshape(1, W2),
            "dis0s1": _f32(dis0s1).reshape(1, W1),
            "u0w": _f32(u0_w),
            "u0br": _f32(u0_b).reshape(H, 1),
            "u1w": _f32(u1_w),
            "u1br": _f32(u1_b).reshape(H, 1),
            "u2w": _f32(u2_w),
            "u2bn": _f32(u2_b).reshape(1, F_IN),
        })
    r4 = _run("ph4", in4)
    z = np.concatenate([r4[c]["z_out"] for c in range(NC)], axis=0)
    return z.astype(np.float32)

